# revision 6
# baseline (speedup 1.0000x reference)
"""CUTSEncoder Trainium2 kernel: 4x conv3x3+BN(train)+LeakyReLU backbone,
patch/latent gathers, recon linear. SPMD over 8 NeuronCores, one image per
core. BN statistics all-reduced across cores; recon weight tensor-sharded
over its output dim.

Self-contained: hardcodes all shapes for the nn_CUTSEncoder problem
(B=8, C=3, H=W=256, K=32, L=256, S=32, P=9).
"""
import sys

sys.path.insert(0, "/opt/trn_rl_repo")

import numpy as np
import ml_dtypes

import concourse.bass as bass
import concourse.mybir as mybir
import concourse.tile as tile
from concourse import bacc
from concourse.bass_utils import run_bass_kernel_spmd
from concourse.masks import make_identity

DT = mybir.dt
F32 = DT.float32
BF16 = DT.bfloat16
I32 = DT.int32

NCORES = 8
RG = [list(range(NCORES))]
H = W = 256
HP = H + 2  # padded 258
HW = H * W
C1, K1, K2, K3, K4 = 3, 32, 64, 128, 256
S = 32
EPS = 1e-5
SLOPE = 0.01
NBLK = H // 2  # 128 blocks of 2 rows (N=512)
NT = float(NCORES * HW)  # batchnorm population count
TAPS = [(ky, kx) for ky in range(3) for kx in range(3)]
AF = mybir.ActivationFunctionType
OP = mybir.AluOpType

_PROG = None  # cached (nc,) built once per process
_LAST_RESULT = None
_LAST_WALL_S = None


def _build():
    nc = bacc.Bacc("TRN2", debug=False, num_devices=NCORES, enable_asserts=False)

    # ---- I/O ----
    xim_d = nc.dram_tensor("xim", [27, HP * HP], BF16, kind="ExternalInput")
    w1_d = nc.dram_tensor("w1t", [27, K1], BF16, kind="ExternalInput")
    w2_d = nc.dram_tensor("w2t", [K1, 9 * K2], BF16, kind="ExternalInput")
    w3_d = nc.dram_tensor("w3t", [K2, 9 * K3], BF16, kind="ExternalInput")
    w4_d = nc.dram_tensor("w4t", [K3, 9 * K4], BF16, kind="ExternalInput")
    bnp_d = nc.dram_tensor("bnp", [128, 10], F32, kind="ExternalInput")
    gidx_d = nc.dram_tensor("gidx", [128, 192], I32, kind="ExternalInput")
    rwt_d = nc.dram_tensor("rwt", [64, 128, 972], BF16, kind="ExternalInput")
    z_d = nc.dram_tensor("z_out", [K4, HW], F32, kind="ExternalOutput")
    za_d = nc.dram_tensor("za_out", [64, K4], F32, kind="ExternalOutput")
    xr_d = nc.dram_tensor("xr_out", [8, 972], F32, kind="ExternalOutput")

    with tile.TileContext(nc) as tc:
        with tc.tile_pool(name="dram", bufs=1, space="DRAM") as drp, \
             tc.tile_pool(name="wpool", bufs=1) as wp, \
             tc.tile_pool(name="b1", bufs=3) as pb1, \
             tc.tile_pool(name="b2", bufs=3) as pb2, \
             tc.tile_pool(name="b3", bufs=3) as pb3, \
             tc.tile_pool(name="stgp", bufs=3) as stgp, \
             tc.tile_pool(name="scrp", bufs=2) as scrp, \
             tc.tile_pool(name="rbp", bufs=3) as rbp, \
             tc.tile_pool(name="zwp", bufs=2) as zwp, \
             tc.tile_pool(name="rwp", bufs=4) as rwp, \
             tc.tile_pool(name="convp", bufs=3, space="PSUM") as convp, \
             tc.tile_pool(name="tailp", bufs=2, space="PSUM") as tailp:

            # ---- internal DRAM ----
            raw1 = drp.tile([K1, HP * HP], BF16, name="raw1")
            raw2 = drp.tile([K2, HP * HP], BF16, name="raw2")
            raw3 = drp.tile([K3, HW], BF16, name="raw3")
            raw4 = drp.tile([K4, HW], BF16, name="raw4")
            cc_in = [drp.tile([K1, 2], F32, name="cc_in1"),
                     drp.tile([K2, 2], F32, name="cc_in2"),
                     drp.tile([K3, 2], F32, name="cc_in3"),
                     drp.tile([128, 4], F32, name="cc_in4")]
            cc_out = [drp.tile([K1, 2], F32, name="cc_out1", addr_space="Shared"),
                      drp.tile([K2, 2], F32, name="cc_out2", addr_space="Shared"),
                      drp.tile([K3, 2], F32, name="cc_out3", addr_space="Shared"),
                      drp.tile([128, 4], F32, name="cc_out4", addr_space="Shared")]
            ag_in = drp.tile([S, K4], BF16, name="ag_in")
            ag_out = drp.tile([8 * S, K4], BF16, name="ag_out", addr_space="Shared")

            raw1v = raw1[:].rearrange("c (h w) -> c h w", w=HP)
            raw2v = raw2[:].rearrange("c (h w) -> c h w", w=HP)
            raw3f = raw3[:].rearrange("c (s o) -> (c s) o", o=1)

            # ---- persistent SBUF ----
            w1sb = wp.tile([27, K1], BF16)
            w2sb = wp.tile([K1, 9 * K2], BF16)
            w3sb = wp.tile([K2, 9 * K3], BF16)
            w4sb = wp.tile([K3, 9 * K4], BF16)
            bnp_sb = wp.tile([128, 10], F32)
            gidx_sb = wp.tile([128, 192], I32)
            ident_f = wp.tile([128, 128], F32)
            ident_b = wp.tile([128, 128], BF16)
            zero_row = wp.tile([128, HP], BF16)
            st = [[wp.tile([128, NBLK], F32, name=f"st_{i}_{j}")
                   for j in range(2)] for i in range(5)]  # stages 1-3 + s4 halves
            st4b = [wp.tile([128, NBLK], F32, name=f"st4b_{j}") for j in range(2)]
            ccs = wp.tile([128, 4], F32)
            gst = wp.tile([128, 4], F32)
            wk = wp.tile([128, 8], F32)
            scales = wp.tile([128, 4], F32)  # col s = scale for stage s+1 (s4: col3=h0)
            shifts = wp.tile([128, 4], F32)
            scale4b = wp.tile([128, 1], F32)  # stage-4 half-1
            shift4b = wp.tile([128, 1], F32)
            eps_t = wp.tile([128, 1], F32)
            G = wp.tile([128, 576], BF16)
            zc = [wp.tile([128, 64], F32, name=f"zc{h}") for h in range(2)]
            za_sb = wp.tile([64, K4], F32)
            za_bf = wp.tile([S, K4], BF16)
            TT = [wp.tile([128, K4], BF16, name=f"TT{i}") for i in range(2)]
            TTt = [wp.tile([128, K4], BF16, name=f"TTt{i}") for i in range(2)]
            xr_sb = wp.tile([8, 972], F32)
            z3n = wp.tile([K3, HP * HP], BF16)
            z3nv = z3n[:].rearrange("c (h w) -> c h w", w=HP)

            nc.sync.dma_start(w1sb[:], w1_d[:, :])
            nc.sync.dma_start(w2sb[:], w2_d[:, :])
            nc.sync.dma_start(w3sb[:], w3_d[:, :])
            nc.sync.dma_start(w4sb[:], w4_d[:, :])
            nc.sync.dma_start(bnp_sb[:], bnp_d[:, :])
            nc.sync.dma_start(gidx_sb[:], gidx_d[:, :])
            make_identity(nc, ident_f[:])
            make_identity(nc, ident_b[:])
            nc.vector.memset(zero_row[:], 0.0)
            nc.vector.memset(eps_t[:], EPS)

            # zero the pad stripes of raw1/raw2 (rows 0,257 and cols 0,257)
            for rv, C in ((raw1v, K1), (raw2v, K2)):
                nc.sync.dma_start(rv[:, 0, :], zero_row[:C, :])
                nc.sync.dma_start(rv[:, HP - 1, :], zero_row[:C, :])
                nc.sync.dma_start(rv[:, :, 0:1],
                                  zero_row[:C, :].rearrange("c (h o) -> c h o", o=1))
                nc.sync.dma_start(rv[:, :, HP - 1:HP],
                                  zero_row[:C, :].rearrange("c (h o) -> c h o", o=1))

            xim3 = xim_d[:, :].rearrange("q (h w) -> q h w", w=HP)

            def stats_block(psum_ap, C, stg_t, s_sum, s_sq, blk):
                """psum -> bf16 staging w/ per-channel sum; sumsq of staging."""
                nc.vector.tensor_scalar(
                    out=stg_t[:C, :], in0=psum_ap, scalar1=1.0, scalar2=0.0,
                    op0=OP.mult, op1=OP.add, accum_out=s_sum[:C, blk:blk + 1])
                scr = scrp.tile([128, 512], BF16, tag="scr")
                nc.vector.scalar_tensor_tensor(
                    out=scr[:C, :], in0=stg_t[:C, :], scalar=1.0, in1=stg_t[:C, :],
                    op0=OP.mult, op1=OP.mult, accum_out=s_sq[:C, blk:blk + 1])

            def finalize_stats(si, C, gcol, bcol, sc_ap, sh_ap, s_sum, s_sq,
                               ncols=2, ccoff=0):
                nc.vector.reduce_sum(out=ccs[:C, ccoff:ccoff + 1], in_=s_sum[:C, :],
                                     axis=mybir.AxisListType.X)
                nc.vector.reduce_sum(out=ccs[:C, ccoff + 1:ccoff + 2],
                                     in_=s_sq[:C, :], axis=mybir.AxisListType.X)
                if ccoff + 2 < ncols:
                    return  # stage-4 first half: wait for second half
                nc.sync.dma_start(cc_in[si][:, :], ccs[:C, 0:ncols])
                nc.gpsimd.collective_compute(
                    "AllReduce", OP.add, replica_groups=RG,
                    ins=[cc_in[si][:].opt()], outs=[cc_out[si][:].opt()])
                nc.sync.dma_start(gst[:C, 0:ncols], cc_out[si][:, :])

            def scale_shift(C, gcol, bcol, sc_ap, sh_ap, sumcol, sqcol):
                nc.scalar.mul(wk[:C, 0:1], gst[:C, sumcol:sumcol + 1], 1.0 / NT)
                nc.scalar.mul(wk[:C, 1:2], gst[:C, sqcol:sqcol + 1], 1.0 / NT)
                nc.vector.tensor_tensor(out=wk[:C, 2:3], in0=wk[:C, 0:1],
                                        in1=wk[:C, 0:1], op=OP.mult)
                nc.vector.tensor_tensor(out=wk[:C, 3:4], in0=wk[:C, 1:2],
                                        in1=wk[:C, 2:3], op=OP.subtract)
                nc.scalar.activation(wk[:C, 4:5], wk[:C, 3:4], AF.Sqrt,
                                     bias=eps_t[:C, 0:1])
                nc.vector.reciprocal(wk[:C, 5:6], wk[:C, 4:5])
                nc.vector.tensor_tensor(out=sc_ap, in0=bnp_sb[:C, gcol:gcol + 1],
                                        in1=wk[:C, 5:6], op=OP.mult)
                nc.vector.tensor_tensor(out=wk[:C, 6:7], in0=wk[:C, 0:1],
                                        in1=sc_ap, op=OP.mult)
                nc.vector.tensor_tensor(out=sh_ap, in0=bnp_sb[:C, bcol:bcol + 1],
                                        in1=wk[:C, 6:7], op=OP.subtract)

            # ================= stage 1 =================
            for blk in range(NBLK):
                h0 = 2 * blk
                xb = pb1.tile([27, 2, HP], BF16, tag="xb")
                nc.sync.dma_start(xb[:], xim3[:, h0 + 1:h0 + 3, :])
                ps = convp.tile([K1, 512], F32, tag="cps")
                nc.tensor.matmul(ps[:], lhsT=w1sb[:, :], rhs=xb[:, :, 1:257],
                                 start=True, stop=True)
                stg = stgp.tile([128, 512], BF16, tag="stg")
                stats_block(ps[:], K1, stg, st[0][0], st[0][1], blk)
                nc.sync.dma_start(raw1v[:, h0 + 1:h0 + 3, 1:257], stg[:K1, :])
            finalize_stats(0, K1, 0, 1, None, None, st[0][0], st[0][1])
            scale_shift(K1, 0, 1, scales[:K1, 0:1], shifts[:K1, 0:1], 0, 1)

            # ================= stage 2 =================
            for blk in range(NBLK):
                h0 = 2 * blk
                bb = pb2.tile([K1, 4, HP], BF16, tag="bb2")
                nc.sync.dma_start(bb[:], raw1v[:, h0:h0 + 4, :])
                vr0 = 1 if blk == 0 else 0
                vr1 = 3 if blk == NBLK - 1 else 4
                nc.scalar.activation(bb[:, vr0:vr1, 1:257], bb[:, vr0:vr1, 1:257],
                                     AF.Lrelu, bias=shifts[:K1, 0:1],
                                     scale=scales[:K1, 0:1], alpha=SLOPE)
                ps = convp.tile([K2, 512], F32, tag="cps")
                for t, (ky, kx) in enumerate(TAPS):
                    nc.tensor.matmul(ps[:], lhsT=w2sb[:, t * K2:(t + 1) * K2],
                                     rhs=bb[:, ky:ky + 2, kx:kx + 256],
                                     start=(t == 0), stop=(t == 8))
                stg = stgp.tile([128, 512], BF16, tag="stg")
                stats_block(ps[:], K2, stg, st[1][0], st[1][1], blk)
                nc.sync.dma_start(raw2v[:, h0 + 1:h0 + 3, 1:257], stg[:K2, :])
            finalize_stats(1, K2, 2, 3, None, None, st[1][0], st[1][1])
            scale_shift(K2, 2, 3, scales[:K2, 1:2], shifts[:K2, 1:2], 0, 1)

            # ================= stage 3 =================
            for blk in range(NBLK):
                h0 = 2 * blk
                bb = pb3.tile([K2, 4, HP], BF16, tag="bb3")
                nc.sync.dma_start(bb[:], raw2v[:, h0:h0 + 4, :])
                vr0 = 1 if blk == 0 else 0
                vr1 = 3 if blk == NBLK - 1 else 4
                nc.scalar.activation(bb[:, vr0:vr1, 1:257], bb[:, vr0:vr1, 1:257],
                                     AF.Lrelu, bias=shifts[:K2, 1:2],
                                     scale=scales[:K2, 1:2], alpha=SLOPE)
                ps = convp.tile([K3, 512], F32, tag="cps")
                for t, (ky, kx) in enumerate(TAPS):
                    nc.tensor.matmul(ps[:], lhsT=w3sb[:, t * K3:(t + 1) * K3],
                                     rhs=bb[:, ky:ky + 2, kx:kx + 256],
                                     start=(t == 0), stop=(t == 8))
                stg = stgp.tile([128, 512], BF16, tag="stg")
                stats_block(ps[:], K3, stg, st[2][0], st[2][1], blk)
                nc.sync.dma_start(raw3[:, blk * 512:(blk + 1) * 512], stg[:K3, :])
            finalize_stats(2, K3, 4, 5, None, None, st[2][0], st[2][1])
            scale_shift(K3, 4, 5, scales[:K3, 2:3], shifts[:K3, 2:3], 0, 1)

            # ---- anchor/positive neighborhood gather from raw3 (192 x [128,3])
            for col in range(192):
                nc.gpsimd.indirect_dma_start(
                    out=G[:, col * 3:(col + 1) * 3], out_offset=None,
                    in_=raw3f,
                    in_offset=bass.IndirectOffsetOnAxis(
                        ap=gidx_sb[:, col:col + 1], axis=0))
            nc.scalar.activation(G[:], G[:], AF.Lrelu, bias=shifts[:K3, 2:3],
                                 scale=scales[:K3, 2:3], alpha=SLOPE)

            # ---- z3n fill (normalize raw3 into padded SBUF) ----
            nc.vector.memset(z3nv[:, 0, :], 0.0)
            nc.vector.memset(z3nv[:, HP - 1, :], 0.0)
            nc.vector.memset(z3nv[:, :, 0:1], 0.0)
            nc.vector.memset(z3nv[:, :, HP - 1:HP], 0.0)
            for blk in range(NBLK):
                h0 = 2 * blk
                rb = rbp.tile([K3, 512], BF16, tag="rb")
                nc.sync.dma_start(rb[:], raw3[:, blk * 512:(blk + 1) * 512])
                nc.scalar.activation(z3nv[:, h0 + 1:h0 + 3, 1:257], rb[:],
                                     AF.Lrelu, bias=shifts[:K3, 2:3],
                                     scale=scales[:K3, 2:3], alpha=SLOPE)

            # ================= stage 4 =================
            for blk in range(NBLK):
                h0 = 2 * blk
                for Hh in range(2):
                    ps = convp.tile([K3, 512], F32, tag="cps")
                    for t, (ky, kx) in enumerate(TAPS):
                        nc.tensor.matmul(
                            ps[:],
                            lhsT=w4sb[:, t * K4 + Hh * 128:t * K4 + Hh * 128 + 128],
                            rhs=z3nv[:, h0 + ky:h0 + ky + 2, kx:kx + 256],
                            start=(t == 0), stop=(t == 8))
                    stg = stgp.tile([128, 512], BF16, tag="stg")
                    stats_block(ps[:], 128, stg,
                                st[3 + Hh][0] if Hh == 0 else st4b[0],
                                st[3 + Hh][1] if Hh == 0 else st4b[1], blk)
                    nc.sync.dma_start(
                        raw4[Hh * 128:(Hh + 1) * 128, blk * 512:(blk + 1) * 512],
                        stg[:, :])
            # stage-4 stats: both halves into one AllReduce [128, 4]
            nc.vector.reduce_sum(out=ccs[:, 0:1], in_=st[3][0][:, :],
                                 axis=mybir.AxisListType.X)
            nc.vector.reduce_sum(out=ccs[:, 1:2], in_=st[3][1][:, :],
                                 axis=mybir.AxisListType.X)
            nc.vector.reduce_sum(out=ccs[:, 2:3], in_=st4b[0][:, :],
                                 axis=mybir.AxisListType.X)
            nc.vector.reduce_sum(out=ccs[:, 3:4], in_=st4b[1][:, :],
                                 axis=mybir.AxisListType.X)
            nc.sync.dma_start(cc_in[3][:, :], ccs[:, 0:4])
            nc.gpsimd.collective_compute(
                "AllReduce", OP.add, replica_groups=RG,
                ins=[cc_in[3][:].opt()], outs=[cc_out[3][:].opt()])
            nc.sync.dma_start(gst[:, 0:4], cc_out[3][:, :])
            scale_shift(128, 6, 8, scales[:, 3:4], shifts[:, 3:4], 0, 1)
            scale_shift(128, 7, 9, scale4b[:, 0:1], shift4b[:, 0:1], 2, 3)

            # ---- readback: raw4 -> normalize -> z_out ----
            for Hh in range(2):
                sc_ap = scales[:, 3:4] if Hh == 0 else scale4b[:, 0:1]
                sh_ap = shifts[:, 3:4] if Hh == 0 else shift4b[:, 0:1]
                for blk in range(NBLK):
                    rb = rbp.tile([128, 512], BF16, tag="rb")
                    nc.sync.dma_start(
                        rb[:], raw4[Hh * 128:(Hh + 1) * 128,
                                    blk * 512:(blk + 1) * 512])
                    zw = zwp.tile([128, 512], F32, tag="zw")
                    nc.scalar.activation(zw[:], rb[:], AF.Lrelu, bias=sh_ap,
                                         scale=sc_ap, alpha=SLOPE)
                    nc.sync.dma_start(
                        z_d[Hh * 128:(Hh + 1) * 128, blk * 512:(blk + 1) * 512],
                        zw[:])

            # ---- za: conv at anchor/positive points from G ----
            Gr = G[:].rearrange("p (j t) -> p t j", t=9)
            for Hh in range(2):
                sc_ap = scales[:, 3:4] if Hh == 0 else scale4b[:, 0:1]
                sh_ap = shifts[:, 3:4] if Hh == 0 else shift4b[:, 0:1]
                zps = tailp.tile([128, 64], F32, tag="tail")
                for t in range(9):
                    nc.tensor.matmul(
                        zps[:],
                        lhsT=w4sb[:, t * K4 + Hh * 128:t * K4 + Hh * 128 + 128],
                        rhs=Gr[:, t, :], start=(t == 0), stop=(t == 8))
                nc.scalar.activation(zc[Hh][:], zps[:], AF.Lrelu, bias=sh_ap,
                                     scale=sc_ap, alpha=SLOPE)
                tps = tailp.tile([64, 128], F32, tag="tail")
                nc.tensor.transpose(tps[:], zc[Hh][:], ident_f[:])
                nc.vector.tensor_copy(za_sb[:, Hh * 128:(Hh + 1) * 128], tps[:])
            nc.sync.dma_start(za_d[:, :], za_sb[:])

            # ---- AllGather anchors, transpose to [c, (b,s)] ----
            nc.vector.tensor_copy(za_bf[:], za_sb[0:S, :])
            nc.sync.dma_start(ag_in[:], za_bf[:])
            nc.gpsimd.collective_compute(
                "AllGather", OP.bypass, replica_groups=RG,
                ins=[ag_in[:].opt()], outs=[ag_out[:].opt()])
            nc.sync.dma_start(TT[0][:], ag_out[0:128, :])
            nc.sync.dma_start(TT[1][:], ag_out[128:256, :])
            for Hh in range(2):
                for q in range(2):
                    tpb = tailp.tile([128, 128], BF16, tag="tail")
                    nc.tensor.transpose(tpb[:], TT[q][:, Hh * 128:(Hh + 1) * 128],
                                        ident_b[:])
                    nc.vector.tensor_copy(TTt[Hh][:, q * 128:(q + 1) * 128],
                                          tpb[:])

            # ---- recon matmul: x_recon shard [8, 972] ----
            xr0 = tailp.tile([8, 512], F32, tag="xr0", bufs=1)
            xr1 = tailp.tile([8, 512], F32, tag="xr1", bufs=1)
            for g in range(64):
                Hh, sa = divmod(g, S)
                rwt_t = rwp.tile([128, 972], BF16, tag="rwt")
                nc.sync.dma_start(rwt_t[:], rwt_d[g, :, :])
                lhs = TTt[Hh][:].rearrange("p (b s) -> p s b", s=S)[:, sa, :]
                nc.tensor.matmul(xr0[:], lhsT=lhs, rhs=rwt_t[:, 0:512],
                                 start=(g == 0), stop=(g == 63))
                nc.tensor.matmul(xr1[:, 0:460], lhsT=lhs, rhs=rwt_t[:, 512:972],
                                 start=(g == 0), stop=(g == 63))
            nc.vector.tensor_copy(xr_sb[:, 0:512], xr0[:])
            nc.vector.tensor_copy(xr_sb[:, 512:972], xr1[:, 0:460])
            nc.sync.dma_start(xr_d[:, :], xr_sb[:])

    nc.compile()
    return nc


def _bf16(x):
    return np.asarray(x, dtype=ml_dtypes.bfloat16)


def _host_prep(x, anchors_hw, positives_hw, w1, w2, w3, w4,
               g1, be1, g2, be2, g3, be3, g4, be4, rw):
    B = x.shape[0]
    in_maps = []
    # im2col27 of padded x, per image
    xpad2 = np.zeros((B, C1, H + 4, W + 4), np.float32)
    xpad2[:, :, 2:H + 2, 2:W + 2] = x
    w1t = _bf16(np.transpose(w1, (2, 3, 1, 0)).reshape(27, K1))
    w2t = _bf16(np.transpose(w2, (1, 2, 3, 0)).reshape(K1, 9 * K2))
    w3t = _bf16(np.transpose(w3, (1, 2, 3, 0)).reshape(K2, 9 * K3))
    w4t = _bf16(np.transpose(w4, (1, 2, 3, 0)).reshape(K3, 9 * K4))
    bnp = np.zeros((128, 10), np.float32)
    bnp[:K1, 0], bnp[:K1, 1] = g1, be1
    bnp[:K2, 2], bnp[:K2, 3] = g2, be2
    bnp[:K3, 4], bnp[:K3, 5] = g3, be3
    bnp[:, 6], bnp[:, 7] = g4[:128], g4[128:]
    bnp[:, 8], bnp[:, 9] = be4[:128], be4[128:]
    rwr = rw.reshape(NCORES, 972, S, 2, 128)  # [core, o, s, H, c_lo]
    for b in range(B):
        xim = np.empty((27, HP, HP), np.float32)
        for t, (ky, kx) in enumerate(TAPS):
            for c in range(C1):
                xim[t * 3 + c] = xpad2[b, c, ky:ky + HP, kx:kx + HP]
        hws = np.concatenate([anchors_hw[b], positives_hw[b]], axis=0)  # [64,2]
        hv = hws[:, 0].astype(np.int64)
        wv = hws[:, 1].astype(np.int64)
        p = np.arange(128, dtype=np.int64)[:, None]
        gidx = np.empty((128, 192), np.int64)
        for j in range(64):
            for r in range(3):
                gidx[:, j * 3 + r] = (p[:, 0] * HW + (hv[j] - 1 + r) * W
                                      + (wv[j] - 1))
        rwt = np.ascontiguousarray(
            np.transpose(rwr[b], (2, 1, 3, 0)).reshape(64, 128, 972))
        in_maps.append(dict(
            xim=_bf16(xim.reshape(27, HP * HP)),
            w1t=w1t, w2t=w2t, w3t=w3t, w4t=w4t, bnp=bnp,
            gidx=gidx.astype(np.int32),
            rwt=_bf16(rwt),
        ))
    return in_maps


def kernel(x, anchors_hw, positives_hw,
           w1, b1, g1, be1, w2, b2, g2, be2,
           w3, b3, g3, be3, w4, b4, g4, be4, rw, rb):
    global _PROG
    # biases b1..b4 are zeros in this problem but fold them anyway: BN's
    # batch-mean subtraction makes conv bias a no-op *except* through the
    # mean/var, where a constant per-channel offset cancels exactly. So a
    # nonzero conv bias never changes the BN+LeakyReLU output; ignore it.
    x = np.asarray(x, np.float32)
    if _PROG is None:
        _PROG = _build()
    nc = _PROG
    in_maps = _host_prep(x, np.asarray(anchors_hw), np.asarray(positives_hw),
                         np.asarray(w1, np.float32), np.asarray(w2, np.float32),
                         np.asarray(w3, np.float32), np.asarray(w4, np.float32),
                         np.asarray(g1, np.float32), np.asarray(be1, np.float32),
                         np.asarray(g2, np.float32), np.asarray(be2, np.float32),
                         np.asarray(g3, np.float32), np.asarray(be3, np.float32),
                         np.asarray(g4, np.float32), np.asarray(be4, np.float32),
                         np.asarray(rw, np.float32))
    import kernel as _k
    import time as _time
    _t0 = _time.time()
    res = run_bass_kernel_spmd(nc, in_maps, core_ids=list(range(NCORES)))
    _k._LAST_WALL_S = _time.time() - _t0
    _k._LAST_RESULT = res
    B = x.shape[0]
    z = np.stack([res.results[k]["z_out"].reshape(K4, H, W) for k in range(B)])
    za = np.stack([res.results[k]["za_out"] for k in range(B)])  # [B, 64, 256]
    z_anchors = np.ascontiguousarray(za[:, :S, :])
    z_positives = np.ascontiguousarray(za[:, S:, :])
    x_recon = np.concatenate([res.results[k]["xr_out"] for k in range(NCORES)],
                             axis=1) + np.asarray(rb, np.float32)[None, :]
    # x_anchors: pure gather of the raw input (host)
    a = np.asarray(anchors_hw)
    x_anchors = np.empty((B, S, C1, 9, 9), np.float32)
    for b in range(B):
        for s in range(S):
            hh, ww = int(a[b, s, 0]), int(a[b, s, 1])
            x_anchors[b, s] = x[b, :, hh - 4:hh + 5, ww - 4:ww + 5]
    x_anchors_flat = x_anchors.reshape(B, -1)
    return (z, x_anchors_flat, x_recon, z_anchors, z_positives)


# revision 7
# speedup vs baseline: 1.0119x; 1.0119x over previous
"""CUTSEncoder Trainium2 kernel: 4x conv3x3+BN(train)+LeakyReLU backbone,
patch/latent gathers, recon linear. SPMD over 8 NeuronCores, one image per
core. BN statistics all-reduced across cores; recon weight tensor-sharded
over its output dim.

Self-contained: hardcodes all shapes for the nn_CUTSEncoder problem
(B=8, C=3, H=W=256, K=32, L=256, S=32, P=9).
"""
import sys

sys.path.insert(0, "/opt/trn_rl_repo")

import numpy as np
import ml_dtypes

import concourse.bass as bass
import concourse.mybir as mybir
import concourse.tile as tile
from concourse import bacc
from concourse.bass_utils import run_bass_kernel_spmd
from concourse.masks import make_identity

DT = mybir.dt
F32 = DT.float32
BF16 = DT.bfloat16
I32 = DT.int32

NCORES = 8
RG = [list(range(NCORES))]
H = W = 256
HP = H + 2  # padded 258
HW = H * W
C1, K1, K2, K3, K4 = 3, 32, 64, 128, 256
S = 32
EPS = 1e-5
SLOPE = 0.01
NBLK = H // 2  # 128 blocks of 2 rows (N=512)
NT = float(NCORES * HW)  # batchnorm population count
TAPS = [(ky, kx) for ky in range(3) for kx in range(3)]
AF = mybir.ActivationFunctionType
OP = mybir.AluOpType

_PROG = None  # cached (nc,) built once per process
_LAST_RESULT = None
_LAST_WALL_S = None


def _build():
    nc = bacc.Bacc("TRN2", debug=False, num_devices=NCORES, enable_asserts=False)

    # ---- I/O ----
    xim_d = nc.dram_tensor("xim", [27, HP * HP], BF16, kind="ExternalInput")
    w1_d = nc.dram_tensor("w1t", [27, K1], BF16, kind="ExternalInput")
    w2_d = nc.dram_tensor("w2t", [K1, 9 * K2], BF16, kind="ExternalInput")
    w3_d = nc.dram_tensor("w3t", [K2, 9 * K3], BF16, kind="ExternalInput")
    w4_d = nc.dram_tensor("w4t", [K3, 9 * K4], BF16, kind="ExternalInput")
    bnp_d = nc.dram_tensor("bnp", [128, 10], F32, kind="ExternalInput")
    gidx_d = nc.dram_tensor("gidx", [128, 192], I32, kind="ExternalInput")
    rwt_d = nc.dram_tensor("rwt", [64, 128, 972], BF16, kind="ExternalInput")
    z_d = nc.dram_tensor("z_out", [K4, HW], F32, kind="ExternalOutput")
    za_d = nc.dram_tensor("za_out", [64, K4], F32, kind="ExternalOutput")
    xr_d = nc.dram_tensor("xr_out", [8, 972], F32, kind="ExternalOutput")

    with tile.TileContext(nc) as tc:
        with tc.tile_pool(name="dram", bufs=1, space="DRAM") as drp, \
             tc.tile_pool(name="wpool", bufs=1) as wp, \
             tc.tile_pool(name="b1", bufs=3) as pb1, \
             tc.tile_pool(name="b2", bufs=3) as pb2, \
             tc.tile_pool(name="b3", bufs=3) as pb3, \
             tc.tile_pool(name="stgp", bufs=3) as stgp, \
             tc.tile_pool(name="scrp", bufs=2) as scrp, \
             tc.tile_pool(name="rbp", bufs=3) as rbp, \
             tc.tile_pool(name="zwp", bufs=2) as zwp, \
             tc.tile_pool(name="rwp", bufs=4) as rwp, \
             tc.tile_pool(name="convp", bufs=3, space="PSUM") as convp, \
             tc.tile_pool(name="tailp", bufs=2, space="PSUM") as tailp:

            # ---- internal DRAM ----
            raw1 = drp.tile([K1, HP * HP], BF16, name="raw1")
            raw2 = drp.tile([K2, HP * HP], BF16, name="raw2")
            raw3 = drp.tile([K3, HW], BF16, name="raw3")
            raw4 = drp.tile([K4, HW], BF16, name="raw4")
            cc_in = [drp.tile([K1, 2], F32, name="cc_in1"),
                     drp.tile([K2, 2], F32, name="cc_in2"),
                     drp.tile([K3, 2], F32, name="cc_in3"),
                     drp.tile([128, 4], F32, name="cc_in4")]
            cc_out = [drp.tile([K1, 2], F32, name="cc_out1", addr_space="Shared"),
                      drp.tile([K2, 2], F32, name="cc_out2", addr_space="Shared"),
                      drp.tile([K3, 2], F32, name="cc_out3", addr_space="Shared"),
                      drp.tile([128, 4], F32, name="cc_out4", addr_space="Shared")]
            ag_in = drp.tile([S, K4], BF16, name="ag_in")
            ag_out = drp.tile([8 * S, K4], BF16, name="ag_out", addr_space="Shared")

            raw1v = raw1[:].rearrange("c (h w) -> c h w", w=HP)
            raw2v = raw2[:].rearrange("c (h w) -> c h w", w=HP)
            raw3f = raw3[:].rearrange("c (s o) -> (c s) o", o=1)

            # ---- persistent SBUF ----
            w1sb = wp.tile([27, K1], BF16)
            w2sb = wp.tile([K1, 9 * K2], BF16)
            w3sb = wp.tile([K2, 9 * K3], BF16)
            w4sb = wp.tile([K3, 9 * K4], BF16)
            bnp_sb = wp.tile([128, 10], F32)
            gidx_sb = wp.tile([128, 192], I32)
            ident_f = wp.tile([128, 128], F32)
            ident_b = wp.tile([128, 128], BF16)
            zero_row = wp.tile([128, HP], BF16)
            st = [[wp.tile([128, NBLK], F32, name=f"st_{i}_{j}")
                   for j in range(2)] for i in range(5)]  # stages 1-3 + s4 halves
            st4b = [wp.tile([128, NBLK], F32, name=f"st4b_{j}") for j in range(2)]
            ccs = wp.tile([128, 4], F32)
            gst = wp.tile([128, 4], F32)
            wk = wp.tile([128, 8], F32)
            scales = wp.tile([128, 4], F32)  # col s = scale for stage s+1 (s4: col3=h0)
            shifts = wp.tile([128, 4], F32)
            scale4b = wp.tile([128, 1], F32)  # stage-4 half-1
            shift4b = wp.tile([128, 1], F32)
            eps_t = wp.tile([128, 1], F32)
            G = wp.tile([128, 576], BF16)
            zc = [wp.tile([128, 64], F32, name=f"zc{h}") for h in range(2)]
            za_sb = wp.tile([64, K4], F32)
            za_bf = wp.tile([S, K4], BF16)
            TT = [wp.tile([128, K4], BF16, name=f"TT{i}") for i in range(2)]
            TTt = [wp.tile([128, K4], BF16, name=f"TTt{i}") for i in range(2)]
            xr_sb = wp.tile([8, 972], F32)
            z3n = wp.tile([K3, HP * HP], BF16)
            z3nv = z3n[:].rearrange("c (h w) -> c h w", w=HP)

            nc.sync.dma_start(w1sb[:], w1_d[:, :])
            nc.sync.dma_start(w2sb[:], w2_d[:, :])
            nc.sync.dma_start(w3sb[:], w3_d[:, :])
            nc.sync.dma_start(w4sb[:], w4_d[:, :])
            nc.sync.dma_start(bnp_sb[:], bnp_d[:, :])
            nc.sync.dma_start(gidx_sb[:], gidx_d[:, :])
            make_identity(nc, ident_f[:])
            make_identity(nc, ident_b[:])
            nc.vector.memset(zero_row[:], 0.0)
            nc.vector.memset(eps_t[:], EPS)

            # zero the pad stripes of raw1/raw2 (rows 0,257 and cols 0,257)
            for rv, C in ((raw1v, K1), (raw2v, K2)):
                nc.sync.dma_start(rv[:, 0, :], zero_row[:C, :])
                nc.sync.dma_start(rv[:, HP - 1, :], zero_row[:C, :])
                nc.sync.dma_start(rv[:, :, 0:1],
                                  zero_row[:C, :].rearrange("c (h o) -> c h o", o=1))
                nc.sync.dma_start(rv[:, :, HP - 1:HP],
                                  zero_row[:C, :].rearrange("c (h o) -> c h o", o=1))

            xim3 = xim_d[:, :].rearrange("q (h w) -> q h w", w=HP)

            def stats_block(psum_ap, C, stg_t, s_sum, s_sq, blk):
                """psum -> bf16 staging w/ per-channel sum; sumsq of staging."""
                nc.vector.tensor_scalar(
                    out=stg_t[:C, :], in0=psum_ap, scalar1=1.0, scalar2=0.0,
                    op0=OP.mult, op1=OP.add, accum_out=s_sum[:C, blk:blk + 1])
                scr = scrp.tile([128, 512], BF16, tag="scr")
                nc.vector.scalar_tensor_tensor(
                    out=scr[:C, :], in0=stg_t[:C, :], scalar=1.0, in1=stg_t[:C, :],
                    op0=OP.mult, op1=OP.mult, accum_out=s_sq[:C, blk:blk + 1])

            def finalize_stats(si, C, gcol, bcol, sc_ap, sh_ap, s_sum, s_sq,
                               ncols=2, ccoff=0):
                nc.vector.reduce_sum(out=ccs[:C, ccoff:ccoff + 1], in_=s_sum[:C, :],
                                     axis=mybir.AxisListType.X)
                nc.vector.reduce_sum(out=ccs[:C, ccoff + 1:ccoff + 2],
                                     in_=s_sq[:C, :], axis=mybir.AxisListType.X)
                if ccoff + 2 < ncols:
                    return  # stage-4 first half: wait for second half
                nc.sync.dma_start(cc_in[si][:, :], ccs[:C, 0:ncols])
                nc.gpsimd.collective_compute(
                    "AllReduce", OP.add, replica_groups=RG,
                    ins=[cc_in[si][:].opt()], outs=[cc_out[si][:].opt()])
                nc.sync.dma_start(gst[:C, 0:ncols], cc_out[si][:, :])

            def scale_shift(C, gcol, bcol, sc_ap, sh_ap, sumcol, sqcol):
                nc.scalar.mul(wk[:C, 0:1], gst[:C, sumcol:sumcol + 1], 1.0 / NT)
                nc.scalar.mul(wk[:C, 1:2], gst[:C, sqcol:sqcol + 1], 1.0 / NT)
                nc.vector.tensor_tensor(out=wk[:C, 2:3], in0=wk[:C, 0:1],
                                        in1=wk[:C, 0:1], op=OP.mult)
                nc.vector.tensor_tensor(out=wk[:C, 3:4], in0=wk[:C, 1:2],
                                        in1=wk[:C, 2:3], op=OP.subtract)
                nc.scalar.activation(wk[:C, 4:5], wk[:C, 3:4], AF.Sqrt,
                                     bias=eps_t[:C, 0:1])
                nc.vector.reciprocal(wk[:C, 5:6], wk[:C, 4:5])
                nc.vector.tensor_tensor(out=sc_ap, in0=bnp_sb[:C, gcol:gcol + 1],
                                        in1=wk[:C, 5:6], op=OP.mult)
                nc.vector.tensor_tensor(out=wk[:C, 6:7], in0=wk[:C, 0:1],
                                        in1=sc_ap, op=OP.mult)
                nc.vector.tensor_tensor(out=sh_ap, in0=bnp_sb[:C, bcol:bcol + 1],
                                        in1=wk[:C, 6:7], op=OP.subtract)

            # ================= stage 1 =================
            for blk in range(NBLK):
                h0 = 2 * blk
                xb = pb1.tile([27, 2, HP], BF16, tag="xb")
                nc.sync.dma_start(xb[:], xim3[:, h0 + 1:h0 + 3, :])
                ps = convp.tile([K1, 512], F32, tag="cps")
                nc.tensor.matmul(ps[:], lhsT=w1sb[:, :], rhs=xb[:, :, 1:257],
                                 start=True, stop=True)
                stg = stgp.tile([128, 512], BF16, tag="stg")
                stats_block(ps[:], K1, stg, st[0][0], st[0][1], blk)
                nc.sync.dma_start(raw1v[:, h0 + 1:h0 + 3, 1:257], stg[:K1, :])
            finalize_stats(0, K1, 0, 1, None, None, st[0][0], st[0][1])
            scale_shift(K1, 0, 1, scales[:K1, 0:1], shifts[:K1, 0:1], 0, 1)

            # ================= stage 2 =================
            for blk in range(NBLK):
                h0 = 2 * blk
                bb = pb2.tile([K1, 4, HP], BF16, tag="bb2")
                nc.sync.dma_start(bb[:], raw1v[:, h0:h0 + 4, :])
                vr0 = 1 if blk == 0 else 0
                vr1 = 3 if blk == NBLK - 1 else 4
                nc.scalar.activation(bb[:, vr0:vr1, 1:257], bb[:, vr0:vr1, 1:257],
                                     AF.Lrelu, bias=shifts[:K1, 0:1],
                                     scale=scales[:K1, 0:1], alpha=SLOPE)
                ps = convp.tile([K2, 512], F32, tag="cps")
                for t, (ky, kx) in enumerate(TAPS):
                    nc.tensor.matmul(ps[:], lhsT=w2sb[:, t * K2:(t + 1) * K2],
                                     rhs=bb[:, ky:ky + 2, kx:kx + 256],
                                     start=(t == 0), stop=(t == 8))
                stg = stgp.tile([128, 512], BF16, tag="stg")
                stats_block(ps[:], K2, stg, st[1][0], st[1][1], blk)
                nc.sync.dma_start(raw2v[:, h0 + 1:h0 + 3, 1:257], stg[:K2, :])
            finalize_stats(1, K2, 2, 3, None, None, st[1][0], st[1][1])
            scale_shift(K2, 2, 3, scales[:K2, 1:2], shifts[:K2, 1:2], 0, 1)

            # ================= stage 3 =================
            for blk in range(NBLK):
                h0 = 2 * blk
                bb = pb3.tile([K2, 4, HP], BF16, tag="bb3")
                nc.sync.dma_start(bb[:], raw2v[:, h0:h0 + 4, :])
                vr0 = 1 if blk == 0 else 0
                vr1 = 3 if blk == NBLK - 1 else 4
                nc.scalar.activation(bb[:, vr0:vr1, 1:257], bb[:, vr0:vr1, 1:257],
                                     AF.Lrelu, bias=shifts[:K2, 1:2],
                                     scale=scales[:K2, 1:2], alpha=SLOPE)
                ps = convp.tile([K3, 512], F32, tag="cps")
                for t, (ky, kx) in enumerate(TAPS):
                    nc.tensor.matmul(ps[:], lhsT=w3sb[:, t * K3:(t + 1) * K3],
                                     rhs=bb[:, ky:ky + 2, kx:kx + 256],
                                     start=(t == 0), stop=(t == 8))
                stg = stgp.tile([128, 512], BF16, tag="stg")
                stats_block(ps[:], K3, stg, st[2][0], st[2][1], blk)
                nc.sync.dma_start(raw3[:, blk * 512:(blk + 1) * 512], stg[:K3, :])
            finalize_stats(2, K3, 4, 5, None, None, st[2][0], st[2][1])
            scale_shift(K3, 4, 5, scales[:K3, 2:3], shifts[:K3, 2:3], 0, 1)

            # ---- anchor/positive neighborhood gather from raw3 (192 x [128,3])
            for col in range(192):
                nc.gpsimd.indirect_dma_start(
                    out=G[:, col * 3:(col + 1) * 3], out_offset=None,
                    in_=raw3f,
                    in_offset=bass.IndirectOffsetOnAxis(
                        ap=gidx_sb[:, col:col + 1], axis=0))
            nc.scalar.activation(G[:], G[:], AF.Lrelu, bias=shifts[:K3, 2:3],
                                 scale=scales[:K3, 2:3], alpha=SLOPE)

            # ---- z3n fill (normalize raw3 into padded SBUF) ----
            nc.vector.memset(z3nv[:, 0, :], 0.0)
            nc.vector.memset(z3nv[:, HP - 1, :], 0.0)
            nc.vector.memset(z3nv[:, :, 0:1], 0.0)
            nc.vector.memset(z3nv[:, :, HP - 1:HP], 0.0)
            for blk in range(NBLK):
                h0 = 2 * blk
                rb = rbp.tile([K3, 512], BF16, tag="rb")
                nc.sync.dma_start(rb[:], raw3[:, blk * 512:(blk + 1) * 512])
                nc.scalar.activation(z3nv[:, h0 + 1:h0 + 3, 1:257], rb[:],
                                     AF.Lrelu, bias=shifts[:K3, 2:3],
                                     scale=scales[:K3, 2:3], alpha=SLOPE)

            # ================= stage 4 =================
            for blk in range(NBLK):
                h0 = 2 * blk
                for Hh in range(2):
                    ps = convp.tile([K3, 512], F32, tag="cps")
                    for t, (ky, kx) in enumerate(TAPS):
                        nc.tensor.matmul(
                            ps[:],
                            lhsT=w4sb[:, t * K4 + Hh * 128:t * K4 + Hh * 128 + 128],
                            rhs=z3nv[:, h0 + ky:h0 + ky + 2, kx:kx + 256],
                            start=(t == 0), stop=(t == 8))
                    stg = stgp.tile([128, 512], BF16, tag="stg")
                    stats_block(ps[:], 128, stg,
                                st[3 + Hh][0] if Hh == 0 else st4b[0],
                                st[3 + Hh][1] if Hh == 0 else st4b[1], blk)
                    nc.sync.dma_start(
                        raw4[Hh * 128:(Hh + 1) * 128, blk * 512:(blk + 1) * 512],
                        stg[:, :])
            # stage-4 stats: both halves into one AllReduce [128, 4]
            nc.vector.reduce_sum(out=ccs[:, 0:1], in_=st[3][0][:, :],
                                 axis=mybir.AxisListType.X)
            nc.vector.reduce_sum(out=ccs[:, 1:2], in_=st[3][1][:, :],
                                 axis=mybir.AxisListType.X)
            nc.vector.reduce_sum(out=ccs[:, 2:3], in_=st4b[0][:, :],
                                 axis=mybir.AxisListType.X)
            nc.vector.reduce_sum(out=ccs[:, 3:4], in_=st4b[1][:, :],
                                 axis=mybir.AxisListType.X)
            nc.sync.dma_start(cc_in[3][:, :], ccs[:, 0:4])
            nc.gpsimd.collective_compute(
                "AllReduce", OP.add, replica_groups=RG,
                ins=[cc_in[3][:].opt()], outs=[cc_out[3][:].opt()])
            nc.sync.dma_start(gst[:, 0:4], cc_out[3][:, :])
            scale_shift(128, 6, 8, scales[:, 3:4], shifts[:, 3:4], 0, 1)
            scale_shift(128, 7, 9, scale4b[:, 0:1], shift4b[:, 0:1], 2, 3)

            # ---- readback: raw4 -> normalize -> z_out ----
            for Hh in range(2):
                sc_ap = scales[:, 3:4] if Hh == 0 else scale4b[:, 0:1]
                sh_ap = shifts[:, 3:4] if Hh == 0 else shift4b[:, 0:1]
                for blk in range(NBLK):
                    rb = rbp.tile([128, 512], BF16, tag="rb")
                    nc.sync.dma_start(
                        rb[:], raw4[Hh * 128:(Hh + 1) * 128,
                                    blk * 512:(blk + 1) * 512])
                    zw = zwp.tile([128, 512], F32, tag="zw")
                    nc.scalar.activation(zw[:], rb[:], AF.Lrelu, bias=sh_ap,
                                         scale=sc_ap, alpha=SLOPE)
                    nc.sync.dma_start(
                        z_d[Hh * 128:(Hh + 1) * 128, blk * 512:(blk + 1) * 512],
                        zw[:])

            # ---- za: conv at anchor/positive points from G ----
            Gr = G[:].rearrange("p (j t) -> p t j", t=9)
            for Hh in range(2):
                sc_ap = scales[:, 3:4] if Hh == 0 else scale4b[:, 0:1]
                sh_ap = shifts[:, 3:4] if Hh == 0 else shift4b[:, 0:1]
                zps = tailp.tile([128, 64], F32, tag="tail")
                for t in range(9):
                    nc.tensor.matmul(
                        zps[:],
                        lhsT=w4sb[:, t * K4 + Hh * 128:t * K4 + Hh * 128 + 128],
                        rhs=Gr[:, t, :], start=(t == 0), stop=(t == 8))
                nc.scalar.activation(zc[Hh][:], zps[:], AF.Lrelu, bias=sh_ap,
                                     scale=sc_ap, alpha=SLOPE)
                tps = tailp.tile([64, 128], F32, tag="tail")
                nc.tensor.transpose(tps[:], zc[Hh][:], ident_f[:])
                nc.vector.tensor_copy(za_sb[:, Hh * 128:(Hh + 1) * 128], tps[:])
            nc.sync.dma_start(za_d[:, :], za_sb[:])

            # ---- AllGather anchors, transpose to [c, (b,s)] ----
            nc.vector.tensor_copy(za_bf[:], za_sb[0:S, :])
            nc.sync.dma_start(ag_in[:], za_bf[:])
            nc.gpsimd.collective_compute(
                "AllGather", OP.bypass, replica_groups=RG,
                ins=[ag_in[:].opt()], outs=[ag_out[:].opt()])
            nc.sync.dma_start(TT[0][:], ag_out[0:128, :])
            nc.sync.dma_start(TT[1][:], ag_out[128:256, :])
            for Hh in range(2):
                for q in range(2):
                    tpb = tailp.tile([128, 128], BF16, tag="tail")
                    nc.tensor.transpose(tpb[:], TT[q][:, Hh * 128:(Hh + 1) * 128],
                                        ident_b[:])
                    nc.vector.tensor_copy(TTt[Hh][:, q * 128:(q + 1) * 128],
                                          tpb[:])

            # ---- recon matmul: x_recon shard [8, 972] ----
            xr0 = tailp.tile([8, 512], F32, tag="xr0", bufs=1)
            xr1 = tailp.tile([8, 512], F32, tag="xr1", bufs=1)
            for g in range(64):
                Hh, sa = divmod(g, S)
                rwt_t = rwp.tile([128, 972], BF16, tag="rwt")
                nc.sync.dma_start(rwt_t[:], rwt_d[g, :, :])
                lhs = TTt[Hh][:].rearrange("p (b s) -> p s b", s=S)[:, sa, :]
                nc.tensor.matmul(xr0[:], lhsT=lhs, rhs=rwt_t[:, 0:512],
                                 start=(g == 0), stop=(g == 63))
                nc.tensor.matmul(xr1[:, 0:460], lhsT=lhs, rhs=rwt_t[:, 512:972],
                                 start=(g == 0), stop=(g == 63))
            nc.vector.tensor_copy(xr_sb[:, 0:512], xr0[:])
            nc.vector.tensor_copy(xr_sb[:, 512:972], xr1[:, 0:460])
            nc.sync.dma_start(xr_d[:, :], xr_sb[:])

    nc.compile()
    return nc


def _bf16(x):
    return np.asarray(x, dtype=ml_dtypes.bfloat16)


def _host_prep(x, anchors_hw, positives_hw, w1, w2, w3, w4,
               g1, be1, g2, be2, g3, be3, g4, be4, rw):
    B = x.shape[0]
    in_maps = []
    # im2col27 of padded x, per image
    xpad2 = np.zeros((B, C1, H + 4, W + 4), np.float32)
    xpad2[:, :, 2:H + 2, 2:W + 2] = x
    w1t = _bf16(np.transpose(w1, (2, 3, 1, 0)).reshape(27, K1))
    w2t = _bf16(np.transpose(w2, (1, 2, 3, 0)).reshape(K1, 9 * K2))
    w3t = _bf16(np.transpose(w3, (1, 2, 3, 0)).reshape(K2, 9 * K3))
    w4t = _bf16(np.transpose(w4, (1, 2, 3, 0)).reshape(K3, 9 * K4))
    bnp = np.zeros((128, 10), np.float32)
    bnp[:K1, 0], bnp[:K1, 1] = g1, be1
    bnp[:K2, 2], bnp[:K2, 3] = g2, be2
    bnp[:K3, 4], bnp[:K3, 5] = g3, be3
    bnp[:, 6], bnp[:, 7] = g4[:128], g4[128:]
    bnp[:, 8], bnp[:, 9] = be4[:128], be4[128:]
    rwr = rw.reshape(NCORES, 972, S, 2, 128)  # [core, o, s, H, c_lo]
    for b in range(B):
        xim = np.empty((27, HP, HP), np.float32)
        for t, (ky, kx) in enumerate(TAPS):
            for c in range(C1):
                xim[t * 3 + c] = xpad2[b, c, ky:ky + HP, kx:kx + HP]
        hws = np.concatenate([anchors_hw[b], positives_hw[b]], axis=0)  # [64,2]
        hv = hws[:, 0].astype(np.int64)
        wv = hws[:, 1].astype(np.int64)
        p = np.arange(128, dtype=np.int64)[:, None]
        gidx = np.empty((128, 192), np.int64)
        for j in range(64):
            for r in range(3):
                gidx[:, j * 3 + r] = (p[:, 0] * HW + (hv[j] - 1 + r) * W
                                      + (wv[j] - 1))
        rwt = np.ascontiguousarray(
            np.transpose(rwr[b], (2, 1, 3, 0)).reshape(64, 128, 972))
        in_maps.append(dict(
            xim=_bf16(xim.reshape(27, HP * HP)),
            w1t=w1t, w2t=w2t, w3t=w3t, w4t=w4t, bnp=bnp,
            gidx=gidx.astype(np.int32),
            rwt=_bf16(rwt),
        ))
    return in_maps


def kernel(x, anchors_hw, positives_hw,
           w1, b1, g1, be1, w2, b2, g2, be2,
           w3, b3, g3, be3, w4, b4, g4, be4, rw, rb):
    global _PROG
    # biases b1..b4 are zeros in this problem but fold them anyway: BN's
    # batch-mean subtraction makes conv bias a no-op *except* through the
    # mean/var, where a constant per-channel offset cancels exactly. So a
    # nonzero conv bias never changes the BN+LeakyReLU output; ignore it.
    x = np.asarray(x, np.float32)
    if _PROG is None:
        _PROG = _build()
    nc = _PROG
    in_maps = _host_prep(x, np.asarray(anchors_hw), np.asarray(positives_hw),
                         np.asarray(w1, np.float32), np.asarray(w2, np.float32),
                         np.asarray(w3, np.float32), np.asarray(w4, np.float32),
                         np.asarray(g1, np.float32), np.asarray(be1, np.float32),
                         np.asarray(g2, np.float32), np.asarray(be2, np.float32),
                         np.asarray(g3, np.float32), np.asarray(be3, np.float32),
                         np.asarray(g4, np.float32), np.asarray(be4, np.float32),
                         np.asarray(rw, np.float32))
    import time as _time
    global _LAST_RESULT, _LAST_WALL_S
    _t0 = _time.time()
    res = run_bass_kernel_spmd(nc, in_maps, core_ids=list(range(NCORES)))
    _LAST_WALL_S = _time.time() - _t0
    _LAST_RESULT = res
    B = x.shape[0]
    z = np.stack([res.results[k]["z_out"].reshape(K4, H, W) for k in range(B)])
    za = np.stack([res.results[k]["za_out"] for k in range(B)])  # [B, 64, 256]
    z_anchors = np.ascontiguousarray(za[:, :S, :])
    z_positives = np.ascontiguousarray(za[:, S:, :])
    x_recon = np.concatenate([res.results[k]["xr_out"] for k in range(NCORES)],
                             axis=1) + np.asarray(rb, np.float32)[None, :]
    # x_anchors: pure gather of the raw input (host)
    a = np.asarray(anchors_hw)
    x_anchors = np.empty((B, S, C1, 9, 9), np.float32)
    for b in range(B):
        for s in range(S):
            hh, ww = int(a[b, s, 0]), int(a[b, s, 1])
            x_anchors[b, s] = x[b, :, hh - 4:hh + 5, ww - 4:ww + 5]
    x_anchors_flat = x_anchors.reshape(B, -1)
    return (z, x_anchors_flat, x_recon, z_anchors, z_positives)
